# revision 9
# baseline (speedup 1.0000x reference)
"""CGCNN on 8 trn2 NeuronCores: full network on-device, data-parallel atoms.

Layout: column-major ("T") feature tiles. Per core d (of 8): atoms
d*12500..(d+1)*12500, padded to 12800 = 100*128 = 25*512. Per conv layer:
  A: u = af @ W2 (rows, bf16, DRAM) + selfT halves = W1{f,c}^T @ afT
  AllGather u -> u_full [100352, 128] (global padded indexing)
  C (pass 1): gated^T tiles = W3^T@nbrT + gathered-u^T (PE transpose-acc)
     + selfT broadcast; accumulate per-feature sum/sumsq.
  D: AllReduce BN1 stats -> s1, b1 (bias folds out of BN exactly).
  E (pass 2): recompute gated^T, f=Sigmoid(s1*g+b1), c=Softplus(...),
     prod, sum over 12 neighbors -> accT; BN2 stats after loop.
  F: AllReduce BN2 stats -> s2, t2.
  G: afT = Softplus(afT + s2*accT + t2), zero pad cols.
Host does: sharding/transposes/bf16 casts, final mean-pool + tiny MLP.
"""
import sys
import numpy as np

sys.path.insert(0, "/opt/trn_rl_repo")
import ml_dtypes

BF16NP = ml_dtypes.bfloat16
FP8NP = ml_dtypes.float8_e4m3
NF_FP8 = True

ATOM_F = 64
NBR_F = 41
ORIG_F = 92
EMB = 128
N_CONV = 3
N_CRYSTALS = 2048
EPS = 1e-5
N_ATOMS = 100000
M = 12
NCORES = 8
ND = N_ATOMS // NCORES          # 12500
NDP = 12800                     # padded to 100*128 = 25*512
NT = NDP // 128                 # 100 tiles
NC512 = NDP // 512              # 25 chunks exactly
NPAD_ALL = NDP * NCORES         # 102400
G = 128                         # gated features (2*ATOM_F)
GH = 64                         # half (filter / core)

_CACHE = {}


def _build():
    import concourse.bacc as bacc
    import concourse.tile as tile
    import concourse.mybir as mybir
    from concourse import bass

    F32 = mybir.dt.float32
    BF16 = mybir.dt.bfloat16
    I32 = mybir.dt.int32
    AF = mybir.ActivationFunctionType
    OP = mybir.AluOpType
    RG = [[0, 1, 2, 3, 4, 5, 6, 7]]

    nc = bacc.Bacc(None, target_bir_lowering=False, debug=False)
    nc.num_devices = NCORES

    # ---- inputs (per-core shards; weights replicated) ----
    xT = nc.dram_tensor("xT", [ORIG_F + 1, NDP], BF16, kind="ExternalInput")
    NFDT = mybir.dt.float8e4 if NF_FP8 else BF16
    nfT = nc.dram_tensor("nfT", [NBR_F, M, NDP], NFDT, kind="ExternalInput")
    idx = nc.dram_tensor("idx", [128, NT * M], I32, kind="ExternalInput")
    inwb = nc.dram_tensor("inwb", [ORIG_F + 1, ATOM_F], BF16, kind="ExternalInput")
    w1f = nc.dram_tensor("w1f", [N_CONV, ATOM_F, GH], BF16, kind="ExternalInput")
    w1c = nc.dram_tensor("w1c", [N_CONV, ATOM_F, GH], BF16, kind="ExternalInput")
    w2 = nc.dram_tensor("w2", [N_CONV, ATOM_F, G], BF16, kind="ExternalInput")
    w3f = nc.dram_tensor("w3f", [N_CONV, NBR_F, GH], NFDT, kind="ExternalInput")
    w3c = nc.dram_tensor("w3c", [N_CONV, NBR_F, GH], NFDT, kind="ExternalInput")
    bn1p = nc.dram_tensor("bn1p", [N_CONV, GH, 4], F32, kind="ExternalInput")
    bn2p = nc.dram_tensor("bn2p", [N_CONV, GH, 2], F32, kind="ExternalInput")
    ident = nc.dram_tensor("ident", [128, 128], BF16, kind="ExternalInput")
    afout = nc.dram_tensor("afout", [ATOM_F, NDP], BF16, kind="ExternalOutput")

    # internal DRAM, reused across layers (Tile tracks cross-layer hazards)
    u_loc_t = nc.dram_tensor("u_loc", [NDP, G], BF16, kind="Internal")
    u_full_t = nc.dram_tensor("u_full", [NPAD_ALL, G], BF16,
                              kind="Internal", addr_space="Shared")
    st1i_t = nc.dram_tensor("st1i", [GH, 4], F32, kind="Internal")
    st1o_t = nc.dram_tensor("st1o", [GH, 4], F32, kind="Internal",
                            addr_space="Shared")
    st2i_t = nc.dram_tensor("st2i", [GH, 2], F32, kind="Internal")
    st2o_t = nc.dram_tensor("st2o", [GH, 2], F32, kind="Internal",
                            addr_space="Shared")
    u_loc = [u_loc_t] * N_CONV
    u_full = [u_full_t] * N_CONV
    st1i = [st1i_t] * N_CONV
    st1o = [st1o_t] * N_CONV
    st2i = [st2i_t] * N_CONV
    st2o = [st2o_t] * N_CONV

    with tile.TileContext(nc) as tc:
        with (
            tc.tile_pool(name="s1", bufs=1) as s1,
            tc.tile_pool(name="sb", bufs=2) as sb,
            tc.tile_pool(name="fat", bufs=1) as fat,
            tc.tile_pool(name="ps", bufs=1, space="PSUM") as ps,
            tc.tile_pool(name="ps2", bufs=2, space="PSUM") as ps2,
        ):
            # ---- persistent SBUF ----
            afT_bf = s1.tile([ATOM_F, NDP], BF16)
            selfTf = s1.tile([GH, NDP], BF16)
            selfTc = s1.tile([GH, NDP], BF16)
            accT = s1.tile([GH, NDP], F32)
            id_t = s1.tile([128, 128], BF16)
            nc.sync.dma_start(out=id_t[:], in_=ident[:, :])
            inwb_t = s1.tile([ORIG_F + 1, ATOM_F], BF16)
            nc.sync.dma_start(out=inwb_t[:], in_=inwb[:, :])
            w1f_t = s1.tile([ATOM_F, N_CONV * GH], BF16)
            w1c_t = s1.tile([ATOM_F, N_CONV * GH], BF16)
            w2_t = s1.tile([ATOM_F, N_CONV * G], BF16)
            for l in range(N_CONV):
                nc.sync.dma_start(out=w1f_t[:, l * GH:(l + 1) * GH], in_=w1f[l, :, :])
                nc.sync.dma_start(out=w1c_t[:, l * GH:(l + 1) * GH], in_=w1c[l, :, :])
                nc.sync.dma_start(out=w2_t[:, l * G:(l + 1) * G], in_=w2[l, :, :])
            w3f_t = s1.tile([NBR_F, N_CONV * GH], NFDT)
            w3c_t = s1.tile([NBR_F, N_CONV * GH], NFDT)
            for l in range(N_CONV):
                nc.sync.dma_start(out=w3f_t[:, l * GH:(l + 1) * GH], in_=w3f[l, :, :])
                nc.sync.dma_start(out=w3c_t[:, l * GH:(l + 1) * GH], in_=w3c[l, :, :])
            bn1p_t = s1.tile([GH, N_CONV * 4], F32)
            bn2p_t = s1.tile([GH, N_CONV * 2], F32)
            for l in range(N_CONV):
                nc.sync.dma_start(out=bn1p_t[:, l * 4:(l + 1) * 4], in_=bn1p[l, :, :])
                nc.sync.dma_start(out=bn2p_t[:, l * 2:(l + 1) * 2], in_=bn2p[l, :, :])
            # staging + stats tiles
            idx_st = s1.tile([128, M], I32)
            stats1 = s1.tile([GH, 4 * NT], F32)      # per-tile sgf, sqf, sgc, sqc
            sq_cols = s1.tile([GH, 8], F32)
            bnv = s1.tile([GH, 12], F32)             # scratch for D/F math
            s1f_t = s1.tile([GH, 1], F32)
            b1f_t = s1.tile([GH, 1], F32)
            s1c_t = s1.tile([GH, 1], F32)
            b1c_t = s1.tile([GH, 1], F32)
            s2_t = s1.tile([GH, 1], F32)
            t2_t = s1.tile([GH, 1], F32)
            ns1f_t = s1.tile([GH, 1], F32)
            nb1f_t = s1.tile([GH, 1], F32)

            for l in range(N_CONV):
                W1f = w1f_t[:, l * GH:(l + 1) * GH]
                W1c = w1c_t[:, l * GH:(l + 1) * GH]
                W2 = w2_t[:, l * G:(l + 1) * G]
                W3f = w3f_t[:, l * GH:(l + 1) * GH]
                W3c = w3c_t[:, l * GH:(l + 1) * GH]

                # ---- A: embed (l==0) + u rows + selfT halves, 512-atom chunks
                with tc.For_i(0, NC512, 1, name=f"A{l}") as i:
                    c512 = bass.ts(i, 512)
                    if l == 0:
                        x_st = sb.tile([ORIG_F + 1, 512], BF16, tag="xst")
                        nc.sync.dma_start(out=x_st[:], in_=xT[:, c512])
                        pe = ps2.tile([ATOM_F, 512], F32, tag="pa")
                        nc.tensor.matmul(pe[:], lhsT=inwb_t[:], rhs=x_st[:],
                                         start=True, stop=True)
                        nc.vector.tensor_copy(out=afT_bf[:, c512], in_=pe[:])
                    # u^T chunk then transpose to rows
                    put = ps2.tile([G, 512], F32, tag="pa")
                    nc.tensor.matmul(put[:], lhsT=W2, rhs=afT_bf[:, c512],
                                     start=True, stop=True)
                    ut_sb = sb.tile([G, 512], BF16, tag="ut")
                    nc.vector.tensor_copy(out=ut_sb[:], in_=put[:])
                    pur = ps2.tile([128, 4, G], F32, tag="pa")
                    for j in range(4):
                        nc.tensor.matmul(pur[:, j, :], lhsT=ut_sb[:, j * 128:(j + 1) * 128],
                                         rhs=id_t[:], start=True, stop=True,
                                         skip_group_check=True)
                    ur_sb = sb.tile([128, 4, G], BF16, tag="ur")
                    nc.vector.tensor_copy(out=ur_sb[:], in_=pur[:])
                    nc.sync.dma_start(
                        out=u_loc[l][:, :].rearrange("(b p) g -> p b g", p=128)[
                            :, bass.ds(i * 4, 4), :],
                        in_=ur_sb[:])
                    # selfT halves
                    psf = ps2.tile([GH, 512], F32, tag="pa")
                    nc.tensor.matmul(psf[:], lhsT=W1f, rhs=afT_bf[:, c512],
                                     start=True, stop=True)
                    nc.vector.tensor_copy(out=selfTf[:, c512], in_=psf[:])
                    psc = ps2.tile([GH, 512], F32, tag="pa")
                    nc.tensor.matmul(psc[:], lhsT=W1c, rhs=afT_bf[:, c512],
                                     start=True, stop=True)
                    nc.vector.tensor_copy(out=selfTc[:, c512], in_=psc[:])

                nc.gpsimd.collective_compute(
                    "AllGather", OP.bypass, replica_groups=RG,
                    ins=[u_loc[l][:].opt()], outs=[u_full[l][:].opt()])

                # ---- C: pass 1 (stats) ----
                with tc.For_i(0, NT, 1, name=f"C{l}") as t:
                    nc.sync.dma_start(out=idx_st[:], in_=idx[:, bass.ts(t, M)])
                    nf_st = sb.tile([NBR_F, M * 128], NFDT, tag="nf")
                    nc.sync.dma_start(out=nf_st[:],
                                      in_=nfT[:, :, bass.ts(t, 128)])
                    pf = ps.tile([GH, M * 128], F32, tag="pf")
                    pc = ps.tile([GH, M * 128], F32, tag="pc")
                    for ch in range(3):
                        c5 = slice(ch * 512, (ch + 1) * 512)
                        nc.tensor.matmul(pf[:, c5], lhsT=W3f, rhs=nf_st[:, c5],
                                         start=True, stop=False,
                                         skip_group_check=True)
                        nc.tensor.matmul(pc[:, c5], lhsT=W3c, rhs=nf_st[:, c5],
                                         start=True, stop=False,
                                         skip_group_check=True)
                    ug = sb.tile([128, M, G], BF16, tag="ug")
                    for m in range(M):
                        nc.gpsimd.indirect_dma_start(
                            out=ug[:, m, :], out_offset=None, in_=u_full[l][:],
                            in_offset=bass.IndirectOffsetOnAxis(
                                ap=idx_st[:, m:m + 1], axis=0))
                    for m in range(M):
                        mb = slice(m * 128, (m + 1) * 128)
                        nc.tensor.matmul(pf[:, mb], lhsT=ug[:, m, 0:GH], rhs=id_t[:],
                                         start=False, stop=(m == M - 1),
                                         skip_group_check=True)
                        nc.tensor.matmul(pc[:, mb], lhsT=ug[:, m, GH:G], rhs=id_t[:],
                                         start=False, stop=(m == M - 1),
                                         skip_group_check=True)
                    selfb_f = selfTf[:, bass.ts(t, 128)].rearrange(
                        "p (o a) -> p o a", o=1).to_broadcast([GH, M, 128])
                    selfb_c = selfTc[:, bass.ts(t, 128)].rearrange(
                        "p (o a) -> p o a", o=1).to_broadcast([GH, M, 128])
                    gtf = fat.tile([GH, M * 128], F32, tag="gtf")
                    gtc = fat.tile([GH, M * 128], F32, tag="gtc")
                    nc.vector.tensor_add(
                        out=gtf[:].rearrange("p (m a) -> p m a", m=M),
                        in0=pf[:].rearrange("p (m a) -> p m a", m=M), in1=selfb_f)
                    nc.vector.tensor_add(
                        out=gtc[:].rearrange("p (m a) -> p m a", m=M),
                        in0=pc[:].rearrange("p (m a) -> p m a", m=M), in1=selfb_c)
                    nc.vector.tensor_reduce(out=stats1[:, bass.ds(t * 4, 1)],
                                            in_=gtf[:], axis=mybir.AxisListType.XYZW,
                                            op=OP.add)
                    nc.vector.tensor_reduce(out=stats1[:, bass.ds(t * 4 + 2, 1)],
                                            in_=gtc[:], axis=mybir.AxisListType.XYZW,
                                            op=OP.add)
                    sqf = fat.tile([GH, M * 128], F32, tag="fq1")
                    nc.scalar.activation(out=sqf[:], in_=gtf[:], func=AF.Square,
                                         accum_out=stats1[:, bass.ds(t * 4 + 1, 1)])
                    sqc = fat.tile([GH, M * 128], F32, tag="fq2")
                    nc.scalar.activation(out=sqc[:], in_=gtc[:], func=AF.Square,
                                         accum_out=stats1[:, bass.ds(t * 4 + 3, 1)])

                # ---- D: BN1 stats allreduce + s1/b1 ----
                red4 = s1.tile([GH, 4], F32, tag=f"red4_{l}")
                nc.vector.tensor_reduce(
                    out=red4[:], in_=stats1[:].rearrange("p (t f) -> p f t", f=4),
                    axis=mybir.AxisListType.X, op=OP.add)
                nc.sync.dma_start(out=st1i[l][:, :], in_=red4[:])
                nc.gpsimd.collective_compute(
                    "AllReduce", OP.add, replica_groups=RG,
                    ins=[st1i[l][:].opt()], outs=[st1o[l][:].opt()])
                ar4 = s1.tile([GH, 4], F32, tag=f"ar4_{l}")
                nc.sync.dma_start(out=ar4[:], in_=st1o[l][:, :])
                inv_n1 = 1.0 / float(N_ATOMS * M)
                BN1 = bn1p_t[:, l * 4:(l + 1) * 4]
                # bnv cols: 0 mean_f, 1 mean_c, 2 var_f, 3 var_c, 4 sd_f, 5 sd_c
                nc.vector.tensor_scalar_mul(out=bnv[:, 0:2],
                                            in0=ar4[:].rearrange("p (f a) -> p a f", a=2)[:, 0, :],
                                            scalar1=inv_n1)
                nc.vector.tensor_scalar_mul(out=bnv[:, 2:4],
                                            in0=ar4[:].rearrange("p (f a) -> p a f", a=2)[:, 1, :],
                                            scalar1=inv_n1)
                nc.vector.tensor_tensor(out=bnv[:, 6:8], in0=bnv[:, 0:2],
                                        in1=bnv[:, 0:2], op=OP.mult)
                nc.vector.tensor_tensor(out=bnv[:, 2:4], in0=bnv[:, 2:4],
                                        in1=bnv[:, 6:8], op=OP.subtract)
                nc.vector.tensor_scalar_add(out=bnv[:, 2:4], in0=bnv[:, 2:4],
                                            scalar1=EPS)
                nc.scalar.activation(out=bnv[:, 4:6], in_=bnv[:, 2:4], func=AF.Ln)
                nc.scalar.activation(out=bnv[:, 4:6], in_=bnv[:, 4:6], func=AF.Exp,
                                     scale=-0.5)
                # s1 = gamma / sd ; b1 = beta - s1 * mean
                nc.vector.tensor_tensor(out=s1f_t[:], in0=BN1[:, 0:1],
                                        in1=bnv[:, 4:5], op=OP.mult)
                nc.vector.tensor_tensor(out=s1c_t[:], in0=BN1[:, 2:3],
                                        in1=bnv[:, 5:6], op=OP.mult)
                nc.vector.tensor_tensor(out=b1f_t[:], in0=s1f_t[:],
                                        in1=bnv[:, 0:1], op=OP.mult)
                nc.vector.tensor_tensor(out=b1f_t[:], in0=BN1[:, 1:2],
                                        in1=b1f_t[:], op=OP.subtract)
                nc.vector.tensor_tensor(out=b1c_t[:], in0=s1c_t[:],
                                        in1=bnv[:, 1:2], op=OP.mult)
                nc.vector.tensor_tensor(out=b1c_t[:], in0=BN1[:, 3:4],
                                        in1=b1c_t[:], op=OP.subtract)
                nc.vector.tensor_scalar_mul(out=ns1f_t[:], in0=s1f_t[:],
                                            scalar1=-1.0)
                nc.vector.tensor_scalar_mul(out=nb1f_t[:], in0=b1f_t[:],
                                            scalar1=-1.0)

                # ---- E: pass 2 ----
                with tc.For_i(0, NT, 1, name=f"E{l}") as t:
                    nc.sync.dma_start(out=idx_st[:], in_=idx[:, bass.ts(t, M)])
                    nf_st = sb.tile([NBR_F, M * 128], NFDT, tag="nf")
                    nc.sync.dma_start(out=nf_st[:],
                                      in_=nfT[:, :, bass.ts(t, 128)])
                    pf = ps.tile([GH, M * 128], F32, tag="pf")
                    pc = ps.tile([GH, M * 128], F32, tag="pc")
                    for ch in range(3):
                        c5 = slice(ch * 512, (ch + 1) * 512)
                        nc.tensor.matmul(pf[:, c5], lhsT=W3f, rhs=nf_st[:, c5],
                                         start=True, stop=False,
                                         skip_group_check=True)
                        nc.tensor.matmul(pc[:, c5], lhsT=W3c, rhs=nf_st[:, c5],
                                         start=True, stop=False,
                                         skip_group_check=True)
                    ug = sb.tile([128, M, G], BF16, tag="ug")
                    for m in range(M):
                        nc.gpsimd.indirect_dma_start(
                            out=ug[:, m, :], out_offset=None, in_=u_full[l][:],
                            in_offset=bass.IndirectOffsetOnAxis(
                                ap=idx_st[:, m:m + 1], axis=0))
                    for m in range(M):
                        mb = slice(m * 128, (m + 1) * 128)
                        nc.tensor.matmul(pf[:, mb], lhsT=ug[:, m, 0:GH], rhs=id_t[:],
                                         start=False, stop=(m == M - 1),
                                         skip_group_check=True)
                        nc.tensor.matmul(pc[:, mb], lhsT=ug[:, m, GH:G], rhs=id_t[:],
                                         start=False, stop=(m == M - 1),
                                         skip_group_check=True)
                    selfb_f = selfTf[:, bass.ts(t, 128)].rearrange(
                        "p (o a) -> p o a", o=1).to_broadcast([GH, M, 128])
                    selfb_c = selfTc[:, bass.ts(t, 128)].rearrange(
                        "p (o a) -> p o a", o=1).to_broadcast([GH, M, 128])
                    gtf = fat.tile([GH, M * 128], F32, tag="gtf")
                    gtc = fat.tile([GH, M * 128], F32, tag="gtc")
                    nc.vector.tensor_add(
                        out=gtf[:].rearrange("p (m a) -> p m a", m=M),
                        in0=pf[:].rearrange("p (m a) -> p m a", m=M), in1=selfb_f)
                    nc.vector.tensor_add(
                        out=gtc[:].rearrange("p (m a) -> p m a", m=M),
                        in0=pc[:].rearrange("p (m a) -> p m a", m=M), in1=selfb_c)
                    fil = fat.tile([GH, M * 128], F32, tag="fq1")
                    cor = fat.tile([GH, M * 128], F32, tag="fq2")
                    # fil = sigmoid(s1f*gtf + b1f) = 1/(1+exp(-(s1f*g+b1f)))
                    nc.scalar.activation(out=fil[:], in_=gtf[:], func=AF.Exp,
                                         bias=nb1f_t[:, 0:1], scale=ns1f_t[:, 0:1])
                    nc.vector.tensor_scalar_add(out=fil[:], in0=fil[:], scalar1=1.0)
                    nc.vector.reciprocal(out=fil[:], in_=fil[:])
                    # cor = softplus(s1c*gtc + b1c) = ln(1+exp(.))
                    nc.scalar.activation(out=cor[:], in_=gtc[:], func=AF.Exp,
                                         bias=b1c_t[:, 0:1], scale=s1c_t[:, 0:1])
                    nc.vector.tensor_scalar_add(out=cor[:], in0=cor[:], scalar1=1.0)
                    prod = fat.tile([GH, M * 128], F32, tag="prod")
                    nc.scalar.activation(out=prod[:], in_=cor[:], func=AF.Ln)
                    nc.vector.tensor_tensor(out=prod[:], in0=fil[:], in1=prod[:],
                                            op=OP.mult)
                    nc.vector.tensor_reduce(
                        out=accT[:, bass.ts(t, 128)],
                        in_=prod[:].rearrange("p (m a) -> p a m", m=M),
                        axis=mybir.AxisListType.X, op=OP.add)

                # ---- F: BN2 stats ----
                nc.vector.memset(accT[:, ND:NDP], 0.0)
                red2 = s1.tile([GH, 2], F32, tag=f"red2_{l}")
                nc.vector.tensor_reduce(out=red2[:, 0:1], in_=accT[:],
                                        axis=mybir.AxisListType.XYZW, op=OP.add)
                for j in range(8):
                    sqa = fat.tile([GH, NDP // 8], F32, tag="prod")
                    nc.scalar.activation(out=sqa[:],
                                         in_=accT[:, j * (NDP // 8):(j + 1) * (NDP // 8)],
                                         func=AF.Square,
                                         accum_out=sq_cols[:, j:j + 1])
                nc.vector.tensor_reduce(out=red2[:, 1:2], in_=sq_cols[:],
                                        axis=mybir.AxisListType.XYZW, op=OP.add)
                nc.sync.dma_start(out=st2i[l][:, :], in_=red2[:])
                nc.gpsimd.collective_compute(
                    "AllReduce", OP.add, replica_groups=RG,
                    ins=[st2i[l][:].opt()], outs=[st2o[l][:].opt()])
                ar2 = s1.tile([GH, 2], F32, tag=f"ar2_{l}")
                nc.sync.dma_start(out=ar2[:], in_=st2o[l][:, :])
                inv_n2 = 1.0 / float(N_ATOMS)
                BN2 = bn2p_t[:, l * 2:(l + 1) * 2]
                nc.vector.tensor_scalar_mul(out=bnv[:, 8:10], in0=ar2[:],
                                            scalar1=inv_n2)
                nc.vector.tensor_tensor(out=bnv[:, 10:11], in0=bnv[:, 8:9],
                                        in1=bnv[:, 8:9], op=OP.mult)
                nc.vector.tensor_tensor(out=bnv[:, 9:10], in0=bnv[:, 9:10],
                                        in1=bnv[:, 10:11], op=OP.subtract)
                nc.vector.tensor_scalar_add(out=bnv[:, 9:10], in0=bnv[:, 9:10],
                                            scalar1=EPS)
                nc.scalar.activation(out=bnv[:, 10:11], in_=bnv[:, 9:10],
                                     func=AF.Ln)
                nc.scalar.activation(out=bnv[:, 10:11], in_=bnv[:, 10:11],
                                     func=AF.Exp, scale=-0.5)
                nc.vector.tensor_tensor(out=s2_t[:], in0=BN2[:, 0:1],
                                        in1=bnv[:, 10:11], op=OP.mult)
                nc.vector.tensor_tensor(out=t2_t[:], in0=s2_t[:],
                                        in1=bnv[:, 8:9], op=OP.mult)
                nc.vector.tensor_tensor(out=t2_t[:], in0=BN2[:, 1:2],
                                        in1=t2_t[:], op=OP.subtract)

                # ---- G: residual + softplus ----
                nc.vector.tensor_scalar(out=accT[:], in0=accT[:],
                                        scalar1=s2_t[:, 0:1], scalar2=t2_t[:, 0:1],
                                        op0=OP.mult, op1=OP.add)
                nc.vector.tensor_add(out=accT[:], in0=accT[:], in1=afT_bf[:])
                nc.scalar.activation(out=accT[:], in_=accT[:], func=AF.Exp)
                nc.vector.tensor_scalar_add(out=accT[:], in0=accT[:], scalar1=1.0)
                nc.scalar.activation(out=afT_bf[:], in_=accT[:], func=AF.Ln)
                nc.vector.memset(afT_bf[:, ND:NDP], 0.0)

            nc.sync.dma_start(out=afout[:, :], in_=afT_bf[:])
    nc.finalize()
    return nc


def _softplus(x):
    return np.log1p(np.exp(-np.abs(x))) + np.maximum(x, 0.0)


def _device_forward(x, nbr_fea, nbr_fea_idx, in_w, in_b, fc_w, bn1_g, bn1_b,
                    bn2_g, bn2_b):
    from concourse.bass_utils import run_bass_kernel_spmd

    if "nc" not in _CACHE:
        _CACHE["nc"] = _build()
    nc = _CACHE["nc"]

    # ---- host prep ----
    F = ATOM_F
    inwb = np.concatenate([in_w, in_b[None, :]], 0).astype(BF16NP)      # [93, 64]
    w1f = np.ascontiguousarray(fc_w[:, :F, :GH]).astype(BF16NP)
    w1c = np.ascontiguousarray(fc_w[:, :F, GH:]).astype(BF16NP)
    w2 = np.ascontiguousarray(fc_w[:, F:2 * F, :]).astype(BF16NP)
    nfdt = FP8NP if NF_FP8 else BF16NP
    w3f = np.ascontiguousarray(fc_w[:, 2 * F:, :GH]).astype(nfdt)
    w3c = np.ascontiguousarray(fc_w[:, 2 * F:, GH:]).astype(nfdt)
    bn1p = np.stack([bn1_g[:, :GH], bn1_b[:, :GH], bn1_g[:, GH:], bn1_b[:, GH:]],
                    axis=2).astype(np.float32)                          # [3, 64, 4]
    bn2p = np.stack([bn2_g, bn2_b], axis=2).astype(np.float32)          # [3, 64, 2]
    ident = np.eye(128, dtype=BF16NP)

    xb = x.astype(BF16NP)
    nb = nbr_fea.astype(nfdt)
    # global padded index remap
    idx_g = ((nbr_fea_idx // ND) * NDP + nbr_fea_idx % ND).astype(np.int32)

    in_maps = []
    for d in range(NCORES):
        sl = slice(d * ND, (d + 1) * ND)
        xT = np.zeros((ORIG_F + 1, NDP), BF16NP)
        xT[:ORIG_F, :ND] = xb[sl].T
        xT[ORIG_F, :ND] = 1.0
        nfT = np.zeros((NBR_F, M, NDP), nfdt)
        nfT[:, :, :ND] = nb[sl].transpose(2, 1, 0)
        idxp = np.full((NDP, M), d * NDP + ND, np.int32)   # pads -> a zero row
        idxp[:ND] = idx_g[sl]
        idxr = np.ascontiguousarray(
            idxp.reshape(NT, 128, M).transpose(1, 0, 2).reshape(128, NT * M))
        in_maps.append({
            "xT": xT, "nfT": nfT, "idx": idxr, "inwb": inwb,
            "w1f": w1f, "w1c": w1c, "w2": w2, "w3f": w3f, "w3c": w3c,
            "bn1p": bn1p, "bn2p": bn2p, "ident": ident,
        })

    r = run_bass_kernel_spmd(nc, in_maps, core_ids=list(range(NCORES)))
    af = np.concatenate(
        [np.asarray(r.results[d]["afout"])[:, :ND].astype(np.float32)
         for d in range(NCORES)], axis=1)                               # [64, N]
    return np.ascontiguousarray(af.T)                                   # [N, 64]


def _host_forward(x, nbr_fea, nbr_fea_idx, in_w, in_b, fc_w, fc_b, bn1_g,
                  bn1_b, bn2_g, bn2_b):
    def _bn(h, g, b):
        mu = h.mean(axis=0)
        var = h.var(axis=0)
        return (h - mu) / np.sqrt(var + EPS) * g + b

    def _sigmoid(v):
        return 1.0 / (1.0 + np.exp(-np.clip(v, -60, 60)))

    atom_fea = x @ in_w + in_b
    n, m = nbr_fea_idx.shape
    f = atom_fea.shape[1]
    for i in range(N_CONV):
        w1 = fc_w[i][:f]
        w2 = fc_w[i][f:2 * f]
        w3 = fc_w[i][2 * f:]
        self_part = atom_fea @ w1
        u = atom_fea @ w2
        gated = u[nbr_fea_idx.reshape(-1)]
        gated += np.repeat(self_part, m, axis=0)
        gated += nbr_fea.reshape(n * m, NBR_F) @ w3
        gated += fc_b[i]
        gated = _bn(gated, bn1_g[i], bn1_b[i])
        prod = _sigmoid(gated[:, :f]) * _softplus(gated[:, f:])
        nbr_sumed = prod.reshape(n, m, f).sum(axis=1)
        nbr_sumed = _bn(nbr_sumed, bn2_g[i], bn2_b[i])
        atom_fea = _softplus(atom_fea + nbr_sumed)
    return atom_fea


def kernel(x, nbr_fea, nbr_fea_idx, batch, in_w, in_b, fc_w, fc_b,
           bn1_g, bn1_b, bn2_g, bn2_b, cf_w, cf_b, out_w, out_b):
    x = np.asarray(x, np.float32)
    nbr_fea = np.asarray(nbr_fea, np.float32)
    nbr_fea_idx = np.asarray(nbr_fea_idx, np.int32)
    batch = np.asarray(batch, np.int32)
    in_w = np.asarray(in_w, np.float32)
    in_b = np.asarray(in_b, np.float32)
    fc_w = np.asarray(fc_w, np.float32)
    fc_b = np.asarray(fc_b, np.float32)
    bn1_g = np.asarray(bn1_g, np.float32)
    bn1_b = np.asarray(bn1_b, np.float32)
    bn2_g = np.asarray(bn2_g, np.float32)
    bn2_b = np.asarray(bn2_b, np.float32)
    cf_w = np.asarray(cf_w, np.float32)
    cf_b = np.asarray(cf_b, np.float32)
    out_w = np.asarray(out_w, np.float32)
    out_b = np.asarray(out_b, np.float32)

    try:
        atom_fea = _device_forward(x, nbr_fea, nbr_fea_idx, in_w, in_b, fc_w,
                                   bn1_g, bn1_b, bn2_g, bn2_b)
    except Exception as e:
        import traceback
        print("DEVICE PATH FAILED, falling back to host:", repr(e),
              file=sys.stderr)
        traceback.print_exc()
        atom_fea = _host_forward(x, nbr_fea, nbr_fea_idx, in_w, in_b, fc_w,
                                 fc_b, bn1_g, bn1_b, bn2_g, bn2_b)

    # ---- mean pool per crystal (batch is sorted) ----
    bounds = np.minimum(np.searchsorted(batch, np.arange(N_CRYSTALS)),
                        len(batch) - 1)
    sums = np.add.reduceat(atom_fea, bounds, axis=0)
    cnts = np.bincount(batch, minlength=N_CRYSTALS).astype(np.float32)
    sums[cnts == 0] = 0.0
    crys_fea = sums / np.maximum(cnts, 1.0)[:, None]
    crys_fea = _softplus(_softplus(crys_fea) @ cf_w + cf_b)
    return (crys_fea @ out_w + out_b).astype(np.float32)


# revision 13
# speedup vs baseline: 1.0851x; 1.0851x over previous
"""CGCNN on 8 trn2 NeuronCores: full network on-device, data-parallel atoms.

Layout: column-major ("T") feature tiles. Per core d (of 8): atoms
d*12500..(d+1)*12500, padded to 12800 = 100*128 = 25*512. Per conv layer:
  A: u = af @ W2 (rows, bf16, DRAM) + selfT halves = W1{f,c}^T @ afT
  AllGather u -> u_full [100352, 128] (global padded indexing)
  C (pass 1): gated^T tiles = W3^T@nbrT + gathered-u^T (PE transpose-acc)
     + selfT broadcast; accumulate per-feature sum/sumsq.
  D: AllReduce BN1 stats -> s1, b1 (bias folds out of BN exactly).
  E (pass 2): recompute gated^T, f=Sigmoid(s1*g+b1), c=Softplus(...),
     prod, sum over 12 neighbors -> accT; BN2 stats after loop.
  F: AllReduce BN2 stats -> s2, t2.
  G: afT = Softplus(afT + s2*accT + t2), zero pad cols.
Host does: sharding/transposes/bf16 casts, final mean-pool + tiny MLP.
"""
import sys
import numpy as np

sys.path.insert(0, "/opt/trn_rl_repo")
import ml_dtypes

try:
    import jax
    jax.config.update("jax_compilation_cache_dir", "/root/.cache/jax_bass_cache")
    jax.config.update("jax_persistent_cache_min_compile_time_secs", 0.0)
except Exception:
    pass

BF16NP = ml_dtypes.bfloat16
FP8NP = ml_dtypes.float8_e4m3
NF_FP8 = True

ATOM_F = 64
NBR_F = 41
ORIG_F = 92
EMB = 128
N_CONV = 3
N_CRYSTALS = 2048
EPS = 1e-5
N_ATOMS = 100000
M = 12
NCORES = 8
ND = N_ATOMS // NCORES          # 12500
NDP = 12800                     # padded to 100*128 = 25*512
NT = NDP // 128                 # 100 tiles
NC512 = NDP // 512              # 25 chunks exactly
NPAD_ALL = NDP * NCORES         # 102400
G = 128                         # gated features (2*ATOM_F)
GH = 64                         # half (filter / core)

_CACHE = {}


def _build():
    import concourse.bacc as bacc
    import concourse.tile as tile
    import concourse.mybir as mybir
    from concourse import bass

    F32 = mybir.dt.float32
    BF16 = mybir.dt.bfloat16
    I32 = mybir.dt.int32
    AF = mybir.ActivationFunctionType
    OP = mybir.AluOpType
    RG = [[0, 1, 2, 3, 4, 5, 6, 7]]

    nc = bacc.Bacc(None, target_bir_lowering=False, debug=False)
    nc.num_devices = NCORES

    # ---- inputs (per-core shards; weights replicated) ----
    xT = nc.dram_tensor("xT", [ORIG_F + 1, NDP], BF16, kind="ExternalInput")
    NFDT = mybir.dt.float8e4 if NF_FP8 else BF16
    nfT = nc.dram_tensor("nfT", [NBR_F, M, NDP], NFDT, kind="ExternalInput")
    idx = nc.dram_tensor("idx", [128, NT * M], I32, kind="ExternalInput")
    inwb = nc.dram_tensor("inwb", [ORIG_F + 1, ATOM_F], BF16, kind="ExternalInput")
    w1f = nc.dram_tensor("w1f", [N_CONV, ATOM_F, GH], BF16, kind="ExternalInput")
    w1c = nc.dram_tensor("w1c", [N_CONV, ATOM_F, GH], BF16, kind="ExternalInput")
    w2 = nc.dram_tensor("w2", [N_CONV, ATOM_F, G], BF16, kind="ExternalInput")
    w3f = nc.dram_tensor("w3f", [N_CONV, NBR_F, GH], NFDT, kind="ExternalInput")
    w3c = nc.dram_tensor("w3c", [N_CONV, NBR_F, GH], NFDT, kind="ExternalInput")
    bn1p = nc.dram_tensor("bn1p", [N_CONV, GH, 4], F32, kind="ExternalInput")
    bn2p = nc.dram_tensor("bn2p", [N_CONV, GH, 2], F32, kind="ExternalInput")
    ident = nc.dram_tensor("ident", [128, 128], BF16, kind="ExternalInput")
    afout = nc.dram_tensor("afout", [ATOM_F, NDP], BF16, kind="ExternalOutput")

    # internal DRAM, reused across layers (Tile tracks cross-layer hazards)
    u_loc_t = nc.dram_tensor("u_loc", [NDP, G], BF16, kind="Internal")
    u_full_t = nc.dram_tensor("u_full", [NPAD_ALL, G], BF16,
                              kind="Internal", addr_space="Shared")
    st1i_t = nc.dram_tensor("st1i", [GH, 4], F32, kind="Internal")
    st1o_t = nc.dram_tensor("st1o", [GH, 4], F32, kind="Internal",
                            addr_space="Shared")
    st2i_t = nc.dram_tensor("st2i", [GH, 2], F32, kind="Internal")
    st2o_t = nc.dram_tensor("st2o", [GH, 2], F32, kind="Internal",
                            addr_space="Shared")
    u_loc = [u_loc_t] * N_CONV
    u_full = [u_full_t] * N_CONV
    st1i = [st1i_t] * N_CONV
    st1o = [st1o_t] * N_CONV
    st2i = [st2i_t] * N_CONV
    st2o = [st2o_t] * N_CONV

    with tile.TileContext(nc) as tc:
        with (
            tc.tile_pool(name="s1", bufs=1) as s1,
            tc.tile_pool(name="sb", bufs=2) as sb,
            tc.tile_pool(name="fat", bufs=1) as fat,
            tc.tile_pool(name="ps", bufs=1, space="PSUM") as ps,
            tc.tile_pool(name="ps2", bufs=2, space="PSUM") as ps2,
        ):
            # ---- persistent SBUF ----
            afT_bf = s1.tile([ATOM_F, NDP], BF16)
            selfTf = s1.tile([GH, NDP], BF16)
            selfTc = s1.tile([GH, NDP], BF16)
            accT = s1.tile([GH, NDP], F32)
            id_t = s1.tile([128, 128], BF16)
            nc.sync.dma_start(out=id_t[:], in_=ident[:, :])
            inwb_t = s1.tile([ORIG_F + 1, ATOM_F], BF16)
            nc.sync.dma_start(out=inwb_t[:], in_=inwb[:, :])
            w1f_t = s1.tile([ATOM_F, N_CONV * GH], BF16)
            w1c_t = s1.tile([ATOM_F, N_CONV * GH], BF16)
            w2_t = s1.tile([ATOM_F, N_CONV * G], BF16)
            for l in range(N_CONV):
                nc.sync.dma_start(out=w1f_t[:, l * GH:(l + 1) * GH], in_=w1f[l, :, :])
                nc.sync.dma_start(out=w1c_t[:, l * GH:(l + 1) * GH], in_=w1c[l, :, :])
                nc.sync.dma_start(out=w2_t[:, l * G:(l + 1) * G], in_=w2[l, :, :])
            w3f_t = s1.tile([NBR_F, N_CONV * GH], NFDT)
            w3c_t = s1.tile([NBR_F, N_CONV * GH], NFDT)
            for l in range(N_CONV):
                nc.sync.dma_start(out=w3f_t[:, l * GH:(l + 1) * GH], in_=w3f[l, :, :])
                nc.sync.dma_start(out=w3c_t[:, l * GH:(l + 1) * GH], in_=w3c[l, :, :])
            bn1p_t = s1.tile([GH, N_CONV * 4], F32)
            bn2p_t = s1.tile([GH, N_CONV * 2], F32)
            for l in range(N_CONV):
                nc.sync.dma_start(out=bn1p_t[:, l * 4:(l + 1) * 4], in_=bn1p[l, :, :])
                nc.sync.dma_start(out=bn2p_t[:, l * 2:(l + 1) * 2], in_=bn2p[l, :, :])
            # staging + stats tiles
            idx_st = s1.tile([128, M], I32)
            stats1 = s1.tile([GH, 4 * NT], F32)      # per-tile sgf, sqf, sgc, sqc
            sq_cols = s1.tile([GH, 8], F32)
            bnv = s1.tile([GH, 12], F32)             # scratch for D/F math
            s1f_t = s1.tile([GH, 1], F32)
            b1f_t = s1.tile([GH, 1], F32)
            s1c_t = s1.tile([GH, 1], F32)
            b1c_t = s1.tile([GH, 1], F32)
            s2_t = s1.tile([GH, 1], F32)
            t2_t = s1.tile([GH, 1], F32)
            ns1f_t = s1.tile([GH, 1], F32)
            nb1f_t = s1.tile([GH, 1], F32)

            for l in range(N_CONV):
                W1f = w1f_t[:, l * GH:(l + 1) * GH]
                W1c = w1c_t[:, l * GH:(l + 1) * GH]
                W2 = w2_t[:, l * G:(l + 1) * G]
                W3f = w3f_t[:, l * GH:(l + 1) * GH]
                W3c = w3c_t[:, l * GH:(l + 1) * GH]

                # ---- A: embed (l==0) + u rows + selfT halves, 512-atom chunks
                with tc.For_i(0, NC512, 1, name=f"A{l}") as i:
                    c512 = bass.ts(i, 512)
                    if l == 0:
                        x_st = sb.tile([ORIG_F + 1, 512], BF16, tag="xst")
                        nc.sync.dma_start(out=x_st[:], in_=xT[:, c512])
                        pe = ps2.tile([ATOM_F, 512], F32, tag="pa")
                        nc.tensor.matmul(pe[:], lhsT=inwb_t[:], rhs=x_st[:],
                                         start=True, stop=True)
                        nc.vector.tensor_copy(out=afT_bf[:, c512], in_=pe[:])
                    # u^T chunk then transpose to rows
                    put = ps2.tile([G, 512], F32, tag="pa")
                    nc.tensor.matmul(put[:], lhsT=W2, rhs=afT_bf[:, c512],
                                     start=True, stop=True)
                    ut_sb = sb.tile([G, 512], BF16, tag="ut")
                    nc.vector.tensor_copy(out=ut_sb[:], in_=put[:])
                    pur = ps2.tile([128, 4, G], F32, tag="pa")
                    for j in range(4):
                        nc.tensor.matmul(pur[:, j, :], lhsT=ut_sb[:, j * 128:(j + 1) * 128],
                                         rhs=id_t[:], start=True, stop=True,
                                         skip_group_check=True)
                    ur_sb = sb.tile([128, 4, G], BF16, tag="ur")
                    nc.vector.tensor_copy(out=ur_sb[:], in_=pur[:])
                    nc.sync.dma_start(
                        out=u_loc[l][:, :].rearrange("(b p) g -> p b g", p=128)[
                            :, bass.ds(i * 4, 4), :],
                        in_=ur_sb[:])
                    # selfT halves
                    psf = ps2.tile([GH, 512], F32, tag="pa")
                    nc.tensor.matmul(psf[:], lhsT=W1f, rhs=afT_bf[:, c512],
                                     start=True, stop=True)
                    nc.vector.tensor_copy(out=selfTf[:, c512], in_=psf[:])
                    psc = ps2.tile([GH, 512], F32, tag="pa")
                    nc.tensor.matmul(psc[:], lhsT=W1c, rhs=afT_bf[:, c512],
                                     start=True, stop=True)
                    nc.vector.tensor_copy(out=selfTc[:, c512], in_=psc[:])

                import os as _os
                if _os.environ.get("KN_NOCOLL"):
                    nc.sync.dma_start(out=u_full[l][0:NDP, :], in_=u_loc[l][:, :])
                else:
                    nc.gpsimd.collective_compute(
                        "AllGather", OP.bypass, replica_groups=RG,
                        ins=[u_loc[l][:].opt()], outs=[u_full[l][:].opt()])

                # ---- C: pass 1 (stats) ----
                with tc.For_i(0, NT, 1, name=f"C{l}") as t:
                    nc.sync.dma_start(out=idx_st[:], in_=idx[:, bass.ts(t, M)])
                    nf_st = sb.tile([NBR_F, M * 128], NFDT, tag="nf")
                    nc.sync.dma_start(out=nf_st[:],
                                      in_=nfT[:, :, bass.ts(t, 128)])
                    pf = ps.tile([GH, M * 128], F32, tag="pf")
                    pc = ps.tile([GH, M * 128], F32, tag="pc")
                    for ch in range(3):
                        c5 = slice(ch * 512, (ch + 1) * 512)
                        nc.tensor.matmul(pf[:, c5], lhsT=W3f, rhs=nf_st[:, c5],
                                         start=True, stop=False,
                                         skip_group_check=True)
                        nc.tensor.matmul(pc[:, c5], lhsT=W3c, rhs=nf_st[:, c5],
                                         start=True, stop=False,
                                         skip_group_check=True)
                    ug = sb.tile([128, M, G], BF16, tag="ug")
                    for m in range(M):
                        nc.gpsimd.indirect_dma_start(
                            out=ug[:, m, :], out_offset=None, in_=u_full[l][:],
                            in_offset=bass.IndirectOffsetOnAxis(
                                ap=idx_st[:, m:m + 1], axis=0))
                    for m in range(M):
                        mb = slice(m * 128, (m + 1) * 128)
                        nc.tensor.matmul(pf[:, mb], lhsT=ug[:, m, 0:GH], rhs=id_t[:],
                                         start=False, stop=(m == M - 1),
                                         skip_group_check=True)
                        nc.tensor.matmul(pc[:, mb], lhsT=ug[:, m, GH:G], rhs=id_t[:],
                                         start=False, stop=(m == M - 1),
                                         skip_group_check=True)
                    selfb_f = selfTf[:, bass.ts(t, 128)].rearrange(
                        "p (o a) -> p o a", o=1).to_broadcast([GH, M, 128])
                    selfb_c = selfTc[:, bass.ts(t, 128)].rearrange(
                        "p (o a) -> p o a", o=1).to_broadcast([GH, M, 128])
                    gtf = fat.tile([GH, M * 128], F32, tag="gtf")
                    gtc = fat.tile([GH, M * 128], F32, tag="gtc")
                    nc.vector.tensor_add(
                        out=gtf[:].rearrange("p (m a) -> p m a", m=M),
                        in0=pf[:].rearrange("p (m a) -> p m a", m=M), in1=selfb_f)
                    nc.vector.tensor_add(
                        out=gtc[:].rearrange("p (m a) -> p m a", m=M),
                        in0=pc[:].rearrange("p (m a) -> p m a", m=M), in1=selfb_c)
                    nc.vector.tensor_reduce(out=stats1[:, bass.ds(t * 4, 1)],
                                            in_=gtf[:], axis=mybir.AxisListType.XYZW,
                                            op=OP.add)
                    nc.vector.tensor_reduce(out=stats1[:, bass.ds(t * 4 + 2, 1)],
                                            in_=gtc[:], axis=mybir.AxisListType.XYZW,
                                            op=OP.add)
                    sqf = fat.tile([GH, M * 128], F32, tag="fq1")
                    nc.scalar.activation(out=sqf[:], in_=gtf[:], func=AF.Square,
                                         accum_out=stats1[:, bass.ds(t * 4 + 1, 1)])
                    sqc = fat.tile([GH, M * 128], F32, tag="fq2")
                    nc.scalar.activation(out=sqc[:], in_=gtc[:], func=AF.Square,
                                         accum_out=stats1[:, bass.ds(t * 4 + 3, 1)])

                # ---- D: BN1 stats allreduce + s1/b1 ----
                red4 = s1.tile([GH, 4], F32, tag=f"red4_{l}")
                nc.vector.tensor_reduce(
                    out=red4[:], in_=stats1[:].rearrange("p (t f) -> p f t", f=4),
                    axis=mybir.AxisListType.X, op=OP.add)
                nc.sync.dma_start(out=st1i[l][:, :], in_=red4[:])
                if _os.environ.get("KN_NOCOLL"):
                    nc.sync.dma_start(out=st1o[l][:, :], in_=st1i[l][:, :])
                else:
                    nc.gpsimd.collective_compute(
                        "AllReduce", OP.add, replica_groups=RG,
                        ins=[st1i[l][:].opt()], outs=[st1o[l][:].opt()])
                ar4 = s1.tile([GH, 4], F32, tag=f"ar4_{l}")
                nc.sync.dma_start(out=ar4[:], in_=st1o[l][:, :])
                inv_n1 = 1.0 / float(N_ATOMS * M)
                BN1 = bn1p_t[:, l * 4:(l + 1) * 4]
                # bnv cols: 0 mean_f, 1 mean_c, 2 var_f, 3 var_c, 4 sd_f, 5 sd_c
                nc.vector.tensor_scalar_mul(out=bnv[:, 0:2],
                                            in0=ar4[:].rearrange("p (f a) -> p a f", a=2)[:, 0, :],
                                            scalar1=inv_n1)
                nc.vector.tensor_scalar_mul(out=bnv[:, 2:4],
                                            in0=ar4[:].rearrange("p (f a) -> p a f", a=2)[:, 1, :],
                                            scalar1=inv_n1)
                nc.vector.tensor_tensor(out=bnv[:, 6:8], in0=bnv[:, 0:2],
                                        in1=bnv[:, 0:2], op=OP.mult)
                nc.vector.tensor_tensor(out=bnv[:, 2:4], in0=bnv[:, 2:4],
                                        in1=bnv[:, 6:8], op=OP.subtract)
                nc.vector.tensor_scalar_add(out=bnv[:, 2:4], in0=bnv[:, 2:4],
                                            scalar1=EPS)
                nc.scalar.activation(out=bnv[:, 4:6], in_=bnv[:, 2:4], func=AF.Ln)
                nc.scalar.activation(out=bnv[:, 4:6], in_=bnv[:, 4:6], func=AF.Exp,
                                     scale=-0.5)
                # s1 = gamma / sd ; b1 = beta - s1 * mean
                nc.vector.tensor_tensor(out=s1f_t[:], in0=BN1[:, 0:1],
                                        in1=bnv[:, 4:5], op=OP.mult)
                nc.vector.tensor_tensor(out=s1c_t[:], in0=BN1[:, 2:3],
                                        in1=bnv[:, 5:6], op=OP.mult)
                nc.vector.tensor_tensor(out=b1f_t[:], in0=s1f_t[:],
                                        in1=bnv[:, 0:1], op=OP.mult)
                nc.vector.tensor_tensor(out=b1f_t[:], in0=BN1[:, 1:2],
                                        in1=b1f_t[:], op=OP.subtract)
                nc.vector.tensor_tensor(out=b1c_t[:], in0=s1c_t[:],
                                        in1=bnv[:, 1:2], op=OP.mult)
                nc.vector.tensor_tensor(out=b1c_t[:], in0=BN1[:, 3:4],
                                        in1=b1c_t[:], op=OP.subtract)
                nc.vector.tensor_scalar_mul(out=ns1f_t[:], in0=s1f_t[:],
                                            scalar1=-1.0)
                nc.vector.tensor_scalar_mul(out=nb1f_t[:], in0=b1f_t[:],
                                            scalar1=-1.0)

                # ---- E: pass 2 ----
                with tc.For_i(0, NT, 1, name=f"E{l}") as t:
                    nc.sync.dma_start(out=idx_st[:], in_=idx[:, bass.ts(t, M)])
                    nf_st = sb.tile([NBR_F, M * 128], NFDT, tag="nf")
                    nc.sync.dma_start(out=nf_st[:],
                                      in_=nfT[:, :, bass.ts(t, 128)])
                    pf = ps.tile([GH, M * 128], F32, tag="pf")
                    pc = ps.tile([GH, M * 128], F32, tag="pc")
                    for ch in range(3):
                        c5 = slice(ch * 512, (ch + 1) * 512)
                        nc.tensor.matmul(pf[:, c5], lhsT=W3f, rhs=nf_st[:, c5],
                                         start=True, stop=False,
                                         skip_group_check=True)
                        nc.tensor.matmul(pc[:, c5], lhsT=W3c, rhs=nf_st[:, c5],
                                         start=True, stop=False,
                                         skip_group_check=True)
                    ug = sb.tile([128, M, G], BF16, tag="ug")
                    for m in range(M):
                        nc.gpsimd.indirect_dma_start(
                            out=ug[:, m, :], out_offset=None, in_=u_full[l][:],
                            in_offset=bass.IndirectOffsetOnAxis(
                                ap=idx_st[:, m:m + 1], axis=0))
                    for m in range(M):
                        mb = slice(m * 128, (m + 1) * 128)
                        nc.tensor.matmul(pf[:, mb], lhsT=ug[:, m, 0:GH], rhs=id_t[:],
                                         start=False, stop=(m == M - 1),
                                         skip_group_check=True)
                        nc.tensor.matmul(pc[:, mb], lhsT=ug[:, m, GH:G], rhs=id_t[:],
                                         start=False, stop=(m == M - 1),
                                         skip_group_check=True)
                    selfb_f = selfTf[:, bass.ts(t, 128)].rearrange(
                        "p (o a) -> p o a", o=1).to_broadcast([GH, M, 128])
                    selfb_c = selfTc[:, bass.ts(t, 128)].rearrange(
                        "p (o a) -> p o a", o=1).to_broadcast([GH, M, 128])
                    gtf = fat.tile([GH, M * 128], F32, tag="gtf")
                    gtc = fat.tile([GH, M * 128], F32, tag="gtc")
                    nc.vector.tensor_add(
                        out=gtf[:].rearrange("p (m a) -> p m a", m=M),
                        in0=pf[:].rearrange("p (m a) -> p m a", m=M), in1=selfb_f)
                    nc.vector.tensor_add(
                        out=gtc[:].rearrange("p (m a) -> p m a", m=M),
                        in0=pc[:].rearrange("p (m a) -> p m a", m=M), in1=selfb_c)
                    fil = fat.tile([GH, M * 128], F32, tag="fq1")
                    cor = fat.tile([GH, M * 128], F32, tag="fq2")
                    # fil = sigmoid(s1f*gtf + b1f) = 1/(1+exp(-(s1f*g+b1f)))
                    nc.scalar.activation(out=fil[:], in_=gtf[:], func=AF.Exp,
                                         bias=nb1f_t[:, 0:1], scale=ns1f_t[:, 0:1])
                    nc.vector.tensor_scalar_add(out=fil[:], in0=fil[:], scalar1=1.0)
                    nc.vector.reciprocal(out=fil[:], in_=fil[:])
                    # cor = softplus(s1c*gtc + b1c) = ln(1+exp(.))
                    nc.scalar.activation(out=cor[:], in_=gtc[:], func=AF.Exp,
                                         bias=b1c_t[:, 0:1], scale=s1c_t[:, 0:1])
                    nc.vector.tensor_scalar_add(out=cor[:], in0=cor[:], scalar1=1.0)
                    prod = fat.tile([GH, M * 128], F32, tag="prod")
                    nc.scalar.activation(out=prod[:], in_=cor[:], func=AF.Ln)
                    nc.vector.tensor_tensor(out=prod[:], in0=fil[:], in1=prod[:],
                                            op=OP.mult)
                    nc.vector.tensor_reduce(
                        out=accT[:, bass.ts(t, 128)],
                        in_=prod[:].rearrange("p (m a) -> p a m", m=M),
                        axis=mybir.AxisListType.X, op=OP.add)

                # ---- F: BN2 stats ----
                nc.vector.memset(accT[:, ND:NDP], 0.0)
                red2 = s1.tile([GH, 2], F32, tag=f"red2_{l}")
                nc.vector.tensor_reduce(out=red2[:, 0:1], in_=accT[:],
                                        axis=mybir.AxisListType.XYZW, op=OP.add)
                for j in range(8):
                    sqa = fat.tile([GH, NDP // 8], F32, tag="prod")
                    nc.scalar.activation(out=sqa[:],
                                         in_=accT[:, j * (NDP // 8):(j + 1) * (NDP // 8)],
                                         func=AF.Square,
                                         accum_out=sq_cols[:, j:j + 1])
                nc.vector.tensor_reduce(out=red2[:, 1:2], in_=sq_cols[:],
                                        axis=mybir.AxisListType.XYZW, op=OP.add)
                nc.sync.dma_start(out=st2i[l][:, :], in_=red2[:])
                if _os.environ.get("KN_NOCOLL"):
                    nc.sync.dma_start(out=st2o[l][:, :], in_=st2i[l][:, :])
                else:
                    nc.gpsimd.collective_compute(
                        "AllReduce", OP.add, replica_groups=RG,
                        ins=[st2i[l][:].opt()], outs=[st2o[l][:].opt()])
                ar2 = s1.tile([GH, 2], F32, tag=f"ar2_{l}")
                nc.sync.dma_start(out=ar2[:], in_=st2o[l][:, :])
                inv_n2 = 1.0 / float(N_ATOMS)
                BN2 = bn2p_t[:, l * 2:(l + 1) * 2]
                nc.vector.tensor_scalar_mul(out=bnv[:, 8:10], in0=ar2[:],
                                            scalar1=inv_n2)
                nc.vector.tensor_tensor(out=bnv[:, 10:11], in0=bnv[:, 8:9],
                                        in1=bnv[:, 8:9], op=OP.mult)
                nc.vector.tensor_tensor(out=bnv[:, 9:10], in0=bnv[:, 9:10],
                                        in1=bnv[:, 10:11], op=OP.subtract)
                nc.vector.tensor_scalar_add(out=bnv[:, 9:10], in0=bnv[:, 9:10],
                                            scalar1=EPS)
                nc.scalar.activation(out=bnv[:, 10:11], in_=bnv[:, 9:10],
                                     func=AF.Ln)
                nc.scalar.activation(out=bnv[:, 10:11], in_=bnv[:, 10:11],
                                     func=AF.Exp, scale=-0.5)
                nc.vector.tensor_tensor(out=s2_t[:], in0=BN2[:, 0:1],
                                        in1=bnv[:, 10:11], op=OP.mult)
                nc.vector.tensor_tensor(out=t2_t[:], in0=s2_t[:],
                                        in1=bnv[:, 8:9], op=OP.mult)
                nc.vector.tensor_tensor(out=t2_t[:], in0=BN2[:, 1:2],
                                        in1=t2_t[:], op=OP.subtract)

                # ---- G: residual + softplus ----
                nc.vector.tensor_scalar(out=accT[:], in0=accT[:],
                                        scalar1=s2_t[:, 0:1], scalar2=t2_t[:, 0:1],
                                        op0=OP.mult, op1=OP.add)
                nc.vector.tensor_add(out=accT[:], in0=accT[:], in1=afT_bf[:])
                nc.scalar.activation(out=accT[:], in_=accT[:], func=AF.Exp)
                nc.vector.tensor_scalar_add(out=accT[:], in0=accT[:], scalar1=1.0)
                nc.scalar.activation(out=afT_bf[:], in_=accT[:], func=AF.Ln)
                nc.vector.memset(afT_bf[:, ND:NDP], 0.0)

            nc.sync.dma_start(out=afout[:, :], in_=afT_bf[:])
    nc.finalize()
    return nc


def _softplus(x):
    return np.log1p(np.exp(-np.abs(x))) + np.maximum(x, 0.0)


def _log(msg):
    import time as _t
    print(f"[kernel {_t.strftime('%H:%M:%S')}] {msg}", file=sys.stderr, flush=True)


def _device_forward(x, nbr_fea, nbr_fea_idx, in_w, in_b, fc_w, bn1_g, bn1_b,
                    bn2_g, bn2_b):
    from concourse.bass_utils import run_bass_kernel_spmd

    _log("build start")
    if "nc" not in _CACHE:
        _CACHE["nc"] = _build()
    nc = _CACHE["nc"]
    _log("build done")

    # ---- host prep ----
    F = ATOM_F
    inwb = np.concatenate([in_w, in_b[None, :]], 0).astype(BF16NP)      # [93, 64]
    w1f = np.ascontiguousarray(fc_w[:, :F, :GH]).astype(BF16NP)
    w1c = np.ascontiguousarray(fc_w[:, :F, GH:]).astype(BF16NP)
    w2 = np.ascontiguousarray(fc_w[:, F:2 * F, :]).astype(BF16NP)
    nfdt = FP8NP if NF_FP8 else BF16NP
    w3f = np.ascontiguousarray(fc_w[:, 2 * F:, :GH]).astype(nfdt)
    w3c = np.ascontiguousarray(fc_w[:, 2 * F:, GH:]).astype(nfdt)
    bn1p = np.stack([bn1_g[:, :GH], bn1_b[:, :GH], bn1_g[:, GH:], bn1_b[:, GH:]],
                    axis=2).astype(np.float32)                          # [3, 64, 4]
    bn2p = np.stack([bn2_g, bn2_b], axis=2).astype(np.float32)          # [3, 64, 2]
    ident = np.eye(128, dtype=BF16NP)

    xb = x.astype(BF16NP)
    nb = nbr_fea.astype(nfdt)
    # global padded index remap
    idx_g = ((nbr_fea_idx // ND) * NDP + nbr_fea_idx % ND).astype(np.int32)

    in_maps = []
    for d in range(NCORES):
        sl = slice(d * ND, (d + 1) * ND)
        xT = np.zeros((ORIG_F + 1, NDP), BF16NP)
        xT[:ORIG_F, :ND] = xb[sl].T
        xT[ORIG_F, :ND] = 1.0
        nfT = np.zeros((NBR_F, M, NDP), nfdt)
        nfT[:, :, :ND] = nb[sl].transpose(2, 1, 0)
        idxp = np.full((NDP, M), d * NDP + ND, np.int32)   # pads -> a zero row
        idxp[:ND] = idx_g[sl]
        idxr = np.ascontiguousarray(
            idxp.reshape(NT, 128, M).transpose(1, 0, 2).reshape(128, NT * M))
        in_maps.append({
            "xT": xT, "nfT": nfT, "idx": idxr, "inwb": inwb,
            "w1f": w1f, "w1c": w1c, "w2": w2, "w3f": w3f, "w3c": w3c,
            "bn1p": bn1p, "bn2p": bn2p, "ident": ident,
        })

    _log("prep done, dispatch start")
    r = run_bass_kernel_spmd(nc, in_maps, core_ids=list(range(NCORES)))
    _log("dispatch done")
    af = np.concatenate(
        [np.asarray(r.results[d]["afout"])[:, :ND].astype(np.float32)
         for d in range(NCORES)], axis=1)                               # [64, N]
    return np.ascontiguousarray(af.T)                                   # [N, 64]


def _host_forward(x, nbr_fea, nbr_fea_idx, in_w, in_b, fc_w, fc_b, bn1_g,
                  bn1_b, bn2_g, bn2_b):
    def _bn(h, g, b):
        mu = h.mean(axis=0)
        var = h.var(axis=0)
        return (h - mu) / np.sqrt(var + EPS) * g + b

    def _sigmoid(v):
        return 1.0 / (1.0 + np.exp(-np.clip(v, -60, 60)))

    atom_fea = x @ in_w + in_b
    n, m = nbr_fea_idx.shape
    f = atom_fea.shape[1]
    for i in range(N_CONV):
        w1 = fc_w[i][:f]
        w2 = fc_w[i][f:2 * f]
        w3 = fc_w[i][2 * f:]
        self_part = atom_fea @ w1
        u = atom_fea @ w2
        gated = u[nbr_fea_idx.reshape(-1)]
        gated += np.repeat(self_part, m, axis=0)
        gated += nbr_fea.reshape(n * m, NBR_F) @ w3
        gated += fc_b[i]
        gated = _bn(gated, bn1_g[i], bn1_b[i])
        prod = _sigmoid(gated[:, :f]) * _softplus(gated[:, f:])
        nbr_sumed = prod.reshape(n, m, f).sum(axis=1)
        nbr_sumed = _bn(nbr_sumed, bn2_g[i], bn2_b[i])
        atom_fea = _softplus(atom_fea + nbr_sumed)
    return atom_fea


def kernel(x, nbr_fea, nbr_fea_idx, batch, in_w, in_b, fc_w, fc_b,
           bn1_g, bn1_b, bn2_g, bn2_b, cf_w, cf_b, out_w, out_b):
    x = np.asarray(x, np.float32)
    nbr_fea = np.asarray(nbr_fea, np.float32)
    nbr_fea_idx = np.asarray(nbr_fea_idx, np.int32)
    batch = np.asarray(batch, np.int32)
    in_w = np.asarray(in_w, np.float32)
    in_b = np.asarray(in_b, np.float32)
    fc_w = np.asarray(fc_w, np.float32)
    fc_b = np.asarray(fc_b, np.float32)
    bn1_g = np.asarray(bn1_g, np.float32)
    bn1_b = np.asarray(bn1_b, np.float32)
    bn2_g = np.asarray(bn2_g, np.float32)
    bn2_b = np.asarray(bn2_b, np.float32)
    cf_w = np.asarray(cf_w, np.float32)
    cf_b = np.asarray(cf_b, np.float32)
    out_w = np.asarray(out_w, np.float32)
    out_b = np.asarray(out_b, np.float32)

    def _plausible(af):
        # post-softplus features: finite, (near-)non-negative, sane spread
        return (af is not None and np.isfinite(af).all() and af.min() > -0.01
                and af.max() < 1e4 and af.std() > 1e-6)

    atom_fea = None
    for attempt in range(2):
        try:
            atom_fea = _device_forward(x, nbr_fea, nbr_fea_idx, in_w, in_b,
                                       fc_w, bn1_g, bn1_b, bn2_g, bn2_b)
        except Exception as e:
            import traceback
            _log(f"device attempt {attempt} raised: {e!r}")
            traceback.print_exc()
            atom_fea = None
        if _plausible(atom_fea):
            break
        _log(f"device attempt {attempt} implausible, retrying")
        atom_fea = None
    if atom_fea is None:
        _log("falling back to host forward")
        atom_fea = _host_forward(x, nbr_fea, nbr_fea_idx, in_w, in_b, fc_w,
                                 fc_b, bn1_g, bn1_b, bn2_g, bn2_b)

    # ---- mean pool per crystal (batch is sorted) ----
    bounds = np.minimum(np.searchsorted(batch, np.arange(N_CRYSTALS)),
                        len(batch) - 1)
    sums = np.add.reduceat(atom_fea, bounds, axis=0)
    cnts = np.bincount(batch, minlength=N_CRYSTALS).astype(np.float32)
    sums[cnts == 0] = 0.0
    crys_fea = sums / np.maximum(cnts, 1.0)[:, None]
    crys_fea = _softplus(_softplus(crys_fea) @ cf_w + cf_b)
    return (crys_fea @ out_w + out_b).astype(np.float32)
